# revision 8
# baseline (speedup 1.0000x reference)
"""Trainium2 Bass kernel for the CNN-VAE loss:

    prob = einsum('klb,hwb->klhw', beta, A) * 5000
    mse  = mean(sum(|x - prob[:, :, None]|^2, axis=1))

Strategy
--------
K*L = 128 == SBUF partition count, so (k,l) lives on partitions and the
40000-pixel hw axis is sharded across the 8 cores (5000 pixels each);
every core sees all 128 (k,l) rows and all 3 channels of its hw slice.

Per core:
  Phase 0 (overlapped with x DMA-in):
    PE:   prob = (5000*beta)^T .T @ A^T in 10 x 500-col fp32 matmuls
          (lhsT = scaled beta^T (3,128) stationary, rhs = A^T (3,500))
    DVE:  copy each PSUM bank into a persistent SBUF prob tile (128,5000)
  Steady state, 5 iterations of 1000 pixels x 3 channels:
    DVE:  d = x - prob  (one (128,3,1000) subtract; prob broadcast over
          the channel dim with a step-0 access pattern)
    ACT:  d2 = Square(d), accum_out -> per-partition partial sum column
  DVE reduces the 5 accum columns to (128,1), DMA'd out.

Host side: shard/transpose inputs, then sum the 8 per-core (128,)
partials and divide by 16*3*200*200 (the mean denominator; the sum over
L is folded into the partition-dim sum).
"""

import numpy as np

K, L, NB, H, W = 16, 8, 3, 200, 200
KL = K * L          # 128 partitions
C = 3               # broadcast channel dim of x
HW = H * W          # 40000
N_CORES = 8
HW_SHARD = HW // N_CORES   # 5000
MCHUNK = 500               # matmul chunk (one PSUM bank)
N_MCHUNKS = HW_SHARD // MCHUNK  # 10
XCHUNK = 1000              # steady-state pixels per iteration
N_X = HW_SHARD // XCHUNK   # 5
SCALE = 5000.0
DENOM = float(K * C * H * W)  # mean over [K, C, H, W] after summing L

_NC = None


def _build():
    global _NC
    if _NC is not None:
        return _NC
    from contextlib import ExitStack

    import concourse.bacc as bacc
    import concourse.mybir as mybir
    import concourse.tile as tile

    f32 = mybir.dt.float32
    nc = bacc.Bacc("TRN2", target_bir_lowering=False, debug=False)

    xs = nc.dram_tensor("xs", [KL, C, HW_SHARD], f32, kind="ExternalInput").ap()
    at = nc.dram_tensor("at", [NB, HW_SHARD], f32, kind="ExternalInput").ap()
    bt = nc.dram_tensor("bt", [NB, KL], f32, kind="ExternalInput").ap()
    out = nc.dram_tensor("out", [KL, 1], f32, kind="ExternalOutput").ap()

    with tile.TileContext(nc) as tc, ExitStack() as ctx:
        const = ctx.enter_context(tc.tile_pool(name="const", bufs=1))
        xpool = ctx.enter_context(tc.tile_pool(name="x", bufs=3))
        dpool = ctx.enter_context(tc.tile_pool(name="d", bufs=2))
        d2pool = ctx.enter_context(tc.tile_pool(name="d2", bufs=2))
        ppool = ctx.enter_context(tc.tile_pool(name="psum", bufs=8, space="PSUM"))

        # Small constant DMAs go first, on the gpsimd SWDGE queue, so they
        # don't queue behind the 7.7MB of x traffic on the sync HWDGE path.
        at_sb = const.tile([NB, HW_SHARD], f32)
        nc.gpsimd.dma_start(at_sb[:], at[:])
        bt_sb = const.tile([NB, KL], f32)
        nc.gpsimd.dma_start(bt_sb[:], bt[:])
        bts = const.tile([NB, KL], f32)
        nc.vector.tensor_scalar_mul(bts[:], bt_sb[:], SCALE)

        # Phase 0: build prob in SBUF via back-to-back matmuls + DVE copies.
        prob_sb = const.tile([KL, HW_SHARD], f32)
        for ci in range(N_MCHUNKS):
            sl = slice(ci * MCHUNK, (ci + 1) * MCHUNK)
            pp = ppool.tile([KL, MCHUNK], f32)
            nc.tensor.matmul(pp[:], bts[:], at_sb[:, sl], start=True, stop=True)
            nc.vector.tensor_copy(prob_sb[:, sl], pp[:])

        # Steady state: subtract + square-accumulate in big fused tiles.
        acc = const.tile([KL, N_X], f32)
        for g in range(N_X):
            sl = slice(g * XCHUNK, (g + 1) * XCHUNK)
            xt = xpool.tile([KL, C, XCHUNK], f32)
            nc.sync.dma_start(xt[:], xs[:, :, sl])
            d = dpool.tile([KL, C, XCHUNK], f32)
            prob_b = prob_sb[:, sl].unsqueeze(1).broadcast_to([KL, C, XCHUNK])
            nc.vector.tensor_sub(d[:], xt[:], prob_b)
            d2 = d2pool.tile([KL, C, XCHUNK], f32)
            nc.scalar.activation(
                d2[:],
                d[:],
                mybir.ActivationFunctionType.Square,
                accum_out=acc[:, g : g + 1],
            )

        red = const.tile([KL, 1], f32)
        nc.vector.tensor_reduce(
            red[:], acc[:], axis=mybir.AxisListType.X, op=mybir.AluOpType.add
        )
        nc.sync.dma_start(out[:], red[:])

    nc.compile()
    _NC = nc
    return nc


def _make_in_maps(x, beta, A):
    x = np.ascontiguousarray(np.asarray(x, dtype=np.float32))
    beta = np.ascontiguousarray(np.asarray(beta, dtype=np.float32))
    A = np.ascontiguousarray(np.asarray(A, dtype=np.float32))

    xr = x.reshape(KL, C, HW)
    at_full = np.ascontiguousarray(A.reshape(HW, NB).T)  # (3, 40000)
    bt = np.ascontiguousarray(beta.reshape(KL, NB).T)    # (3, 128)

    in_maps = []
    for i in range(N_CORES):
        sl = slice(i * HW_SHARD, (i + 1) * HW_SHARD)
        in_maps.append(
            {
                "xs": np.ascontiguousarray(xr[:, :, sl]),
                "at": np.ascontiguousarray(at_full[:, sl]),
                "bt": bt,
            }
        )
    return in_maps


def _run(in_maps, trace=False, **kwargs):
    from concourse import bass_utils

    nc = _build()
    return bass_utils.run_bass_kernel_spmd(
        nc, in_maps, list(range(N_CORES)), trace=trace, **kwargs
    )


def _combine(results):
    total = 0.0
    for r in results:
        total += float(np.sum(np.asarray(r["out"], dtype=np.float64)))
    return np.float32(total / DENOM)


def kernel(x, beta, A):
    res = _run(_make_in_maps(x, beta, A))
    return _combine(res.results)


# revision 9
# speedup vs baseline: 1.4167x; 1.4167x over previous
"""Trainium2 Bass kernel for the CNN-VAE loss:

    prob = einsum('klb,hwb->klhw', beta, A) * 5000
    mse  = mean(sum(|x - prob[:, :, None]|^2, axis=1))

Strategy
--------
K*L = 128 == SBUF partition count, so (k,l) lives on partitions and the
40000-pixel hw axis is sharded across the 8 cores (5000 pixels each);
every core sees all 128 (k,l) rows and all 3 channels of its hw slice.

Per core, pipelined over 10 chunks of 500 pixels:
  PE:   prob chunk = (5000*beta)^T .T @ A^T chunk -> one PSUM bank
        (lhsT = scaled beta^T (3,128) stationary, rhs = A^T (3,500))
  DVE:  d = x - prob   (one (128,3,500) subtract per chunk; the PSUM
        prob tile is broadcast over the channel dim with a step-0 AP)
  ACT:  d = Square(d) in place, accum_out -> per-partition sum column
The (128,10) accumulator is DMA'd out; the host sums partials across
columns, partitions, and cores, and divides by 16*3*200*200 (the mean
denominator; the sum over L is folded into the partition-dim sum).

The two tiny constant DMAs (A^T, beta^T) are forced to complete before
the 7.7MB x stream is issued, so the matmul pipeline starts immediately.
"""

import numpy as np

K, L, NB, H, W = 16, 8, 3, 200, 200
KL = K * L          # 128 partitions
C = 3               # broadcast channel dim of x
HW = H * W          # 40000
N_CORES = 8
HW_SHARD = HW // N_CORES   # 5000
MCHUNK = 500               # chunk size (one PSUM bank)
N_MCHUNKS = HW_SHARD // MCHUNK  # 10
SCALE = 5000.0
DENOM = float(K * C * H * W)  # mean over [K, C, H, W] after summing L

_NC = None


def _build():
    global _NC
    if _NC is not None:
        return _NC
    from contextlib import ExitStack

    import concourse.bacc as bacc
    import concourse.mybir as mybir
    import concourse.tile as tile
    from concourse.bass import _add_dep_helper

    f32 = mybir.dt.float32
    nc = bacc.Bacc("TRN2", target_bir_lowering=False, debug=False)

    xs = nc.dram_tensor("xs", [KL, C, HW_SHARD], f32, kind="ExternalInput").ap()
    at = nc.dram_tensor("at", [NB, HW_SHARD], f32, kind="ExternalInput").ap()
    bt = nc.dram_tensor("bt", [NB, KL], f32, kind="ExternalInput").ap()
    out = nc.dram_tensor("out", [KL, N_MCHUNKS], f32, kind="ExternalOutput").ap()

    with tile.TileContext(nc) as tc, ExitStack() as ctx:
        const = ctx.enter_context(tc.tile_pool(name="const", bufs=1))
        xpool = ctx.enter_context(tc.tile_pool(name="x", bufs=4))
        dpool = ctx.enter_context(tc.tile_pool(name="d", bufs=3))
        ppool = ctx.enter_context(tc.tile_pool(name="psum", bufs=8, space="PSUM"))

        at_sb = const.tile([NB, HW_SHARD], f32)
        at_dma = nc.sync.dma_start(at_sb[:], at[:])
        bt_sb = const.tile([NB, KL], f32)
        bt_dma = nc.sync.dma_start(bt_sb[:], bt[:])
        bts = const.tile([NB, KL], f32)
        nc.vector.tensor_scalar_mul(bts[:], bt_sb[:], SCALE)

        acc = const.tile([KL, N_MCHUNKS], f32)

        for ci in range(N_MCHUNKS):
            sl = slice(ci * MCHUNK, (ci + 1) * MCHUNK)
            pp = ppool.tile([KL, MCHUNK], f32)
            nc.tensor.matmul(pp[:], bts[:], at_sb[:, sl], start=True, stop=True)
            xt = xpool.tile([KL, C, MCHUNK], f32)
            xd = nc.sync.dma_start(xt[:], xs[:, :, sl])
            if ci == 0:
                # Let the 60KB constants win the DMA-engine race before the
                # x flood starts; costs ~1us of x latency, saves ~6 on PE.
                _add_dep_helper(xd.ins, at_dma.ins, sync=True, reason="consts first")
                _add_dep_helper(xd.ins, bt_dma.ins, sync=True, reason="consts first")
            d = dpool.tile([KL, C, MCHUNK], f32)
            prob_b = pp[:].unsqueeze(1).broadcast_to([KL, C, MCHUNK])
            nc.vector.tensor_sub(d[:], xt[:], prob_b)
            nc.scalar.activation(
                d[:],
                d[:],
                mybir.ActivationFunctionType.Square,
                accum_out=acc[:, ci : ci + 1],
            )

        nc.sync.dma_start(out[:], acc[:])

    nc.compile()
    _NC = nc
    return nc


def _make_in_maps(x, beta, A):
    x = np.ascontiguousarray(np.asarray(x, dtype=np.float32))
    beta = np.ascontiguousarray(np.asarray(beta, dtype=np.float32))
    A = np.ascontiguousarray(np.asarray(A, dtype=np.float32))

    xr = x.reshape(KL, C, HW)
    at_full = np.ascontiguousarray(A.reshape(HW, NB).T)  # (3, 40000)
    bt = np.ascontiguousarray(beta.reshape(KL, NB).T)    # (3, 128)

    in_maps = []
    for i in range(N_CORES):
        sl = slice(i * HW_SHARD, (i + 1) * HW_SHARD)
        in_maps.append(
            {
                "xs": np.ascontiguousarray(xr[:, :, sl]),
                "at": np.ascontiguousarray(at_full[:, sl]),
                "bt": bt,
            }
        )
    return in_maps


def _run(in_maps, trace=False, **kwargs):
    from concourse import bass_utils

    nc = _build()
    return bass_utils.run_bass_kernel_spmd(
        nc, in_maps, list(range(N_CORES)), trace=trace, **kwargs
    )


def _combine(results):
    total = 0.0
    for r in results:
        total += float(np.sum(np.asarray(r["out"], dtype=np.float64)))
    return np.float32(total / DENOM)


def kernel(x, beta, A):
    res = _run(_make_in_maps(x, beta, A))
    return _combine(res.results)


# revision 11
# speedup vs baseline: 1.4242x; 1.0053x over previous
"""Trainium2 Bass kernel for the CNN-VAE loss:

    prob = einsum('klb,hwb->klhw', beta, A) * 5000
    mse  = mean(sum(|x - prob[:, :, None]|^2, axis=1))

Strategy
--------
K*L = 128 == SBUF partition count, so (k,l) lives on partitions and the
40000-pixel hw axis is sharded across the 8 cores (5000 pixels each);
every core sees all 128 (k,l) rows and all 3 channels of its hw slice.

Per core, pipelined over 10 chunks of 500 pixels:
  PE:   prob chunk = (5000*beta)^T .T @ A^T chunk -> one PSUM bank
        (lhsT = scaled beta^T (3,128) stationary, rhs = A^T (3,500))
  DVE:  d = x - prob   (one (128,3,500) subtract per chunk; the PSUM
        prob tile is broadcast over the channel dim with a step-0 AP)
  ACT:  d = Square(d) in place, accum_out -> per-partition sum column
The (128,10) accumulator is DMA'd out; the host sums partials across
columns, partitions, and cores, and divides by 16*3*200*200 (the mean
denominator; the sum over L is folded into the partition-dim sum).

The two tiny constant DMAs (A^T, beta^T) are forced to complete before
the 7.7MB x stream is issued, so the matmul pipeline starts immediately.
"""

import numpy as np

K, L, NB, H, W = 16, 8, 3, 200, 200
KL = K * L          # 128 partitions
C = 3               # broadcast channel dim of x
HW = H * W          # 40000
N_CORES = 8
HW_SHARD = HW // N_CORES   # 5000
MCHUNK = 500               # chunk size (one PSUM bank)
N_MCHUNKS = HW_SHARD // MCHUNK  # 10
SCALE = 5000.0
DENOM = float(K * C * H * W)  # mean over [K, C, H, W] after summing L

_NC = None


def _build():
    global _NC
    if _NC is not None:
        return _NC
    from contextlib import ExitStack

    import concourse.bacc as bacc
    import concourse.mybir as mybir
    import concourse.tile as tile

    f32 = mybir.dt.float32
    nc = bacc.Bacc("TRN2", target_bir_lowering=False, debug=False)

    xs = nc.dram_tensor("xs", [KL, C, HW_SHARD], f32, kind="ExternalInput").ap()
    at = nc.dram_tensor("at", [NB, HW_SHARD], f32, kind="ExternalInput").ap()
    bt = nc.dram_tensor("bt", [NB, KL], f32, kind="ExternalInput").ap()
    out = nc.dram_tensor("out", [KL, N_MCHUNKS], f32, kind="ExternalOutput").ap()

    with tile.TileContext(nc) as tc, ExitStack() as ctx:
        const = ctx.enter_context(tc.tile_pool(name="const", bufs=1))
        xpool = ctx.enter_context(tc.tile_pool(name="x", bufs=4))
        dpool = ctx.enter_context(tc.tile_pool(name="d", bufs=3))
        ppool = ctx.enter_context(tc.tile_pool(name="psum", bufs=8, space="PSUM"))

        at_sb = const.tile([NB, HW_SHARD], f32)
        nc.sync.dma_start(at_sb[:], at[:])
        bt_sb = const.tile([NB, KL], f32)
        nc.sync.dma_start(bt_sb[:], bt[:])
        bts = const.tile([NB, KL], f32)
        nc.vector.tensor_scalar_mul(bts[:], bt_sb[:], SCALE)

        acc = const.tile([KL, N_MCHUNKS], f32)

        for ci in range(N_MCHUNKS):
            sl = slice(ci * MCHUNK, (ci + 1) * MCHUNK)
            pp = ppool.tile([KL, MCHUNK], f32)
            nc.tensor.matmul(pp[:], bts[:], at_sb[:, sl], start=True, stop=True)
            xt = xpool.tile([KL, C, MCHUNK], f32)
            nc.sync.dma_start(xt[:], xs[:, :, sl])
            d = dpool.tile([KL, C, MCHUNK], f32)
            prob_b = pp[:].unsqueeze(1).broadcast_to([KL, C, MCHUNK])
            nc.vector.tensor_sub(d[:], xt[:], prob_b)
            nc.scalar.activation(
                d[:],
                d[:],
                mybir.ActivationFunctionType.Square,
                accum_out=acc[:, ci : ci + 1],
            )

        nc.sync.dma_start(out[:], acc[:])

    nc.compile()
    _NC = nc
    return nc


def _make_in_maps(x, beta, A):
    x = np.ascontiguousarray(np.asarray(x, dtype=np.float32))
    beta = np.ascontiguousarray(np.asarray(beta, dtype=np.float32))
    A = np.ascontiguousarray(np.asarray(A, dtype=np.float32))

    xr = x.reshape(KL, C, HW)
    at_full = np.ascontiguousarray(A.reshape(HW, NB).T)  # (3, 40000)
    bt = np.ascontiguousarray(beta.reshape(KL, NB).T)    # (3, 128)

    in_maps = []
    for i in range(N_CORES):
        sl = slice(i * HW_SHARD, (i + 1) * HW_SHARD)
        in_maps.append(
            {
                "xs": np.ascontiguousarray(xr[:, :, sl]),
                "at": np.ascontiguousarray(at_full[:, sl]),
                "bt": bt,
            }
        )
    return in_maps


def _run(in_maps, trace=False, **kwargs):
    from concourse import bass_utils

    nc = _build()
    return bass_utils.run_bass_kernel_spmd(
        nc, in_maps, list(range(N_CORES)), trace=trace, **kwargs
    )


def _combine(results):
    total = 0.0
    for r in results:
        total += float(np.sum(np.asarray(r["out"], dtype=np.float64)))
    return np.float32(total / DENOM)


def kernel(x, beta, A):
    res = _run(_make_in_maps(x, beta, A))
    return _combine(res.results)


# revision 13
# speedup vs baseline: 1.4470x; 1.0160x over previous
"""Trainium2 Bass kernel for the CNN-VAE loss:

    prob = einsum('klb,hwb->klhw', beta, A) * 5000
    mse  = mean(sum(|x - prob[:, :, None]|^2, axis=1))

Strategy
--------
K*L = 128 == SBUF partition count, so (k,l) lives on partitions and the
40000-pixel hw axis is sharded across the 8 cores (5000 pixels each);
every core sees all 128 (k,l) rows and all 3 channels of its hw slice.

Per core, pipelined over 5 groups of 1000 pixels:
  PE:   prob group = (5000*beta)^T .T @ A^T, two 500-col fp32 matmuls
        into the two banks of a (128,1000) PSUM tile
        (lhsT = scaled beta^T (3,128) stationary, rhs = A^T (3,500))
  DVE:  x -= prob  (one in-place (128,3,1000) subtract per group; the
        PSUM prob tile is broadcast over the channel dim via a step-0 AP)
  ACT:  x = Square(x) in place, accum_out -> per-partition sum column
The (128,5) accumulator is DMA'd out; the host sums partials across
columns, partitions, and cores, and divides by 16*3*200*200 (the mean
denominator; the sum over L is folded into the partition-dim sum).

A^T and beta^T are concatenated into a single (3, 5128) constant input
so one early DMA delivers both before the 7.7MB x stream saturates the
DMA engines.
"""

import numpy as np

K, L, NB, H, W = 16, 8, 3, 200, 200
KL = K * L          # 128 partitions
C = 3               # broadcast channel dim of x
HW = H * W          # 40000
N_CORES = 8
HW_SHARD = HW // N_CORES   # 5000
MCHUNK = 500               # matmul chunk (one PSUM bank)
GROUP = 1000               # pixels per steady-state iteration
N_GROUPS = HW_SHARD // GROUP    # 5
CONST_W = HW_SHARD + KL    # 5128: A^T shard columns + beta^T columns
SCALE = 5000.0
DENOM = float(K * C * H * W)  # mean over [K, C, H, W] after summing L

_NC = None


def _build():
    global _NC
    if _NC is not None:
        return _NC
    from contextlib import ExitStack

    import concourse.bacc as bacc
    import concourse.mybir as mybir
    import concourse.tile as tile

    f32 = mybir.dt.float32
    nc = bacc.Bacc("TRN2", target_bir_lowering=False, debug=False)

    xs = nc.dram_tensor("xs", [KL, C, HW_SHARD], f32, kind="ExternalInput").ap()
    cb = nc.dram_tensor("cb", [NB, CONST_W], f32, kind="ExternalInput").ap()
    out = nc.dram_tensor("out", [KL, N_GROUPS], f32, kind="ExternalOutput").ap()

    with tile.TileContext(nc) as tc, ExitStack() as ctx:
        const = ctx.enter_context(tc.tile_pool(name="const", bufs=1))
        xpool = ctx.enter_context(tc.tile_pool(name="x", bufs=4))
        ppool = ctx.enter_context(tc.tile_pool(name="psum", bufs=4, space="PSUM"))

        cb_sb = const.tile([NB, CONST_W], f32)
        nc.sync.dma_start(cb_sb[:], cb[:])
        bts = const.tile([NB, KL], f32)
        nc.vector.tensor_scalar_mul(bts[:], cb_sb[:, HW_SHARD:CONST_W], SCALE)

        acc = const.tile([KL, N_GROUPS], f32)

        BANK = 512  # PSUM bank width in f32; matmul output must stay in-bank
        for g in range(N_GROUPS):
            pp = ppool.tile([KL, 2 * BANK], f32)  # two PSUM banks
            for h in range(GROUP // MCHUNK):
                ci = g * (GROUP // MCHUNK) + h
                nc.tensor.matmul(
                    pp[:, h * BANK : h * BANK + MCHUNK],
                    bts[:],
                    cb_sb[:, ci * MCHUNK : (ci + 1) * MCHUNK],
                    start=True,
                    stop=True,
                )
            xt = xpool.tile([KL, C, GROUP], f32)
            nc.sync.dma_start(xt[:], xs[:, :, g * GROUP : (g + 1) * GROUP])
            pv = pp[:].rearrange("p (u f) -> p u f", f=BANK)[:, :, :MCHUNK]
            prob_b = pv.unsqueeze(1).broadcast_to([KL, C, 2, MCHUNK])
            xv = xt[:].rearrange("p c (u f) -> p c u f", f=MCHUNK)
            nc.vector.tensor_sub(xv, xv, prob_b)
            nc.scalar.activation(
                xt[:],
                xt[:],
                mybir.ActivationFunctionType.Square,
                accum_out=acc[:, g : g + 1],
            )

        nc.sync.dma_start(out[:], acc[:])

    nc.compile()
    _NC = nc
    return nc


def _make_in_maps(x, beta, A):
    x = np.ascontiguousarray(np.asarray(x, dtype=np.float32))
    beta = np.ascontiguousarray(np.asarray(beta, dtype=np.float32))
    A = np.ascontiguousarray(np.asarray(A, dtype=np.float32))

    xr = x.reshape(KL, C, HW)
    at_full = A.reshape(HW, NB).T          # (3, 40000)
    bt = beta.reshape(KL, NB).T            # (3, 128)

    in_maps = []
    for i in range(N_CORES):
        sl = slice(i * HW_SHARD, (i + 1) * HW_SHARD)
        cb = np.concatenate([at_full[:, sl], bt], axis=1)  # (3, 5128)
        in_maps.append(
            {
                "xs": np.ascontiguousarray(xr[:, :, sl]),
                "cb": np.ascontiguousarray(cb),
            }
        )
    return in_maps


def _run(in_maps, trace=False, **kwargs):
    from concourse import bass_utils

    nc = _build()
    return bass_utils.run_bass_kernel_spmd(
        nc, in_maps, list(range(N_CORES)), trace=trace, **kwargs
    )


def _combine(results):
    total = 0.0
    for r in results:
        total += float(np.sum(np.asarray(r["out"], dtype=np.float64)))
    return np.float32(total / DENOM)


def kernel(x, beta, A):
    res = _run(_make_in_maps(x, beta, A))
    return _combine(res.results)
